# revision 21
# baseline (speedup 1.0000x reference)
"""Trainium2 Bass kernel for 16-head MHA (B=4, S=1024, D=1024).

Returns (out, attention) matching the reference:
    Q/K/V = x @ W.T + b  (per-head split), scores = QK^T/sqrt(64),
    attention = softmax(scores), out = (attention @ V concat) @ Wo.T + bo.

Sharding: 8 cores = 4 batches x 2 head-groups (8 heads each).
Each core computes its batch's projections restricted to its 512
head-dims, full attention for its 8 heads, and a partial output
projection (contraction over its 512 head-dims). Host sums the two
partials per batch and adds bo.

Per-core layout (all matmul operands bf16, accumulation fp32):
  - inputs arrive transposed: xT[d, s] so projections need no on-chip
    transposes: QT/KT[c, s] (head-dim on partitions), V[s, c].
  - scores both ways: S[q, k] for the softmax/attention output (row
    softmax along free dim, exp row-sums via activation accum_out) and
    ST[k, q] for the P@V matmul (contraction needs k on partitions).
  - OT[c, q] = V^T @ exp(ST) accumulated over k blocks, normalized by
    broadcast 1/rowsum, then the output projection contracts c blocks.
"""

from contextlib import ExitStack

import ml_dtypes
import numpy as np

B = 4
S = 1024
D = 1024
H = 16
DH = 64
P = 128
C = 512  # head dims per core
HPC = 8  # heads per core
ND = D // P
NS = S // P
NCB = C // P
SCALE = 0.125  # 1 / sqrt(DH)

_CACHE = {}

# walrus in this environment rejects sequencer CTRL instructions (Drain/NoOp)
# carrying more than a couple of sem waits ("Too many sync wait commands").
# TileContext's tail drain waits on every active proc sem, so split it into a
# chain of drains with at most _MAX_CTRL_WAITS waits each.
_MAX_CTRL_WAITS = 1


def _patch_tail_drain():
    import concourse.mybir as mybir
    import concourse.tile as tile
    from concourse.vector_clock import ScopedClock

    if getattr(tile.TileContext._drain_and_barrier, "_split_waits", False):
        return

    def _drain_and_barrier(self, tick_clock, wait_clock):
        nc = self.nc
        drain_inst = nc.sync.drain()
        wait_clock.add_sem_waits(
            drain_inst.ins, ScopedClock({None: tick_clock.global_clock})
        )
        si = drain_inst.ins.sync_info
        waits = list(si.on_wait) if si is not None else []
        if len(waits) > _MAX_CTRL_WAITS:
            drain_inst.ins.sync_info = mybir.SyncInfo(
                on_wait=waits[:_MAX_CTRL_WAITS],
                on_update=list(si.on_update),
            )
            for i in range(_MAX_CTRL_WAITS, len(waits), _MAX_CTRL_WAITS):
                extra = nc.sync.drain()
                extra.ins.sync_info = mybir.SyncInfo(
                    on_wait=waits[i : i + _MAX_CTRL_WAITS], on_update=[]
                )
        nc.all_engine_barrier()
        assert self.sems is not None
        popped = nc._tile_sem_poison_stack.pop()
        assert popped is self._sem_poison
        nc.clear_and_free_semaphores(list(self.sems.allocated().values()))
        nc.all_engine_barrier()

    _drain_and_barrier._split_waits = True
    tile.TileContext._drain_and_barrier = _drain_and_barrier


def _split_sync_waits(nc, mybir, max_waits=1):
    """walrus here allows only one sync-wait command per instruction; hoist
    extra waits onto same-engine NoOps inserted just before the instruction
    (engine streams execute in block order, so this is equivalent)."""
    for fn in nc.m.functions:
        for blk in fn.blocks:
            out = []
            changed = False
            for inst in blk.instructions:
                si = inst.sync_info
                waits = list(si.on_wait) if si is not None else []
                if len(waits) > max_waits:
                    changed = True
                    for j, w in enumerate(waits[max_waits:]):
                        nop = mybir.InstNoOp(
                            name=f"{inst.name}-w{j}", ins=[], outs=[]
                        )
                        nop.engine = inst.engine
                        nop.sync_info = mybir.SyncInfo(on_wait=[w], on_update=[])
                        nc.register_instruction(nop, overwrite=True)
                        out.append(nop)
                    inst.sync_info = mybir.SyncInfo(
                        on_wait=waits[:max_waits], on_update=list(si.on_update)
                    )
                out.append(inst)
            if changed:
                blk.instructions = out


def _build_nc():
    import concourse.bass as bass
    import concourse.mybir as mybir
    import concourse.tile as tile

    _patch_tail_drain()

    FP32 = mybir.dt.float32
    BF16 = mybir.dt.bfloat16
    AF = mybir.ActivationFunctionType

    nc = bass.Bass("TRN2", target_bir_lowering=False, debug=False)

    xqT_d = nc.dram_tensor("xqT", [D, S], BF16, kind="ExternalInput")
    xkT_d = nc.dram_tensor("xkT", [D, S], BF16, kind="ExternalInput")
    xvT_d = nc.dram_tensor("xvT", [D, S], BF16, kind="ExternalInput")
    wqT_d = nc.dram_tensor("wqT", [D, C], BF16, kind="ExternalInput")
    wkT_d = nc.dram_tensor("wkT", [D, C], BF16, kind="ExternalInput")
    wvT_d = nc.dram_tensor("wvT", [D, C], BF16, kind="ExternalInput")
    bq_d = nc.dram_tensor("bq", [1, C], BF16, kind="ExternalInput")
    bk_d = nc.dram_tensor("bk", [1, C], BF16, kind="ExternalInput")
    bv_d = nc.dram_tensor("bv", [1, C], BF16, kind="ExternalInput")
    woT_d = nc.dram_tensor("woT", [C, D], BF16, kind="ExternalInput")
    attn_d = nc.dram_tensor("attn", [HPC, S, S], FP32, kind="ExternalOutput")
    part_d = nc.dram_tensor("partial", [S, D], FP32, kind="ExternalOutput")

    with tile.TileContext(nc) as tc:
        with ExitStack() as ctx:
            _body(
                ctx,
                tc,
                nc,
                FP32,
                BF16,
                AF,
                xqT_d,
                xkT_d,
                xvT_d,
                wqT_d,
                wkT_d,
                wvT_d,
                bq_d,
                bk_d,
                bv_d,
                woT_d,
                attn_d,
                part_d,
            )
    _split_sync_waits(nc, mybir)
    return nc


def _body(
    ctx,
    tc,
    nc,
    FP32,
    BF16,
    AF,
    xqT_d,
    xkT_d,
    xvT_d,
    wqT_d,
    wkT_d,
    wvT_d,
    bq_d,
    bk_d,
    bv_d,
    woT_d,
    attn_d,
    part_d,
):
    persist = ctx.enter_context(tc.tile_pool(name="persist", bufs=1))

    def ptile(name, shape, dt):
        return persist.tile(shape, dt, name=name)

    xq = [ptile(f"xq{i}", [P, S], BF16) for i in range(ND)]
    xk = [ptile(f"xk{i}", [P, S], BF16) for i in range(ND)]
    xv = [ptile(f"xv{i}", [P, S], BF16) for i in range(ND)]
    wq = [ptile(f"wq{i}", [P, C], BF16) for i in range(ND)]
    wk = [ptile(f"wk{i}", [P, C], BF16) for i in range(ND)]
    wv = [ptile(f"wv{i}", [P, C], BF16) for i in range(ND)]
    wo = [ptile(f"wo{i}", [DH, D], BF16) for i in range(HPC)]
    bqs = ptile("bqs", [1, C], BF16)
    bks = ptile("bks", [1, C], BF16)
    bvs = ptile("bvs", [1, C], BF16)
    ones = ptile("ones", [1, S], BF16)
    QT = [ptile(f"qt{i}", [P, S], BF16) for i in range(NCB)]
    KT = [ptile(f"kt{i}", [P, S], BF16) for i in range(NCB)]
    V = [ptile(f"v{i}", [P, C], BF16) for i in range(NS)]
    OT = [ptile(f"otc{i}", [DH, S], BF16) for i in range(HPC)]
    r_all = ptile("r_all", [P, HPC * NS], FP32)
    rinv_all = ptile("rinv_all", [P, HPC * NS], FP32)

    dma = nc.sync.dma_start
    tdma = nc.scalar.dma_start_transpose

    for i in range(ND):
        dma(xq[i][:], xqT_d.ap()[i * P : (i + 1) * P, :])
        dma(xk[i][:], xkT_d.ap()[i * P : (i + 1) * P, :])
        dma(xv[i][:], xvT_d.ap()[i * P : (i + 1) * P, :])
        dma(wq[i][:], wqT_d.ap()[i * P : (i + 1) * P, :])
        dma(wk[i][:], wkT_d.ap()[i * P : (i + 1) * P, :])
        dma(wv[i][:], wvT_d.ap()[i * P : (i + 1) * P, :])
    for i in range(HPC):
        dma(wo[i][:], woT_d.ap()[i * DH : (i + 1) * DH, :])
    dma(bqs[:], bq_d.ap()[:, :])
    dma(bks[:], bk_d.ap()[:, :])
    dma(bvs[:], bv_d.ap()[:, :])
    nc.vector.memset(ones[:], 1.0)

    # ---- projections: QT[c,s], KT[c,s] (transposed), V[s,c] (natural) ----
    with ExitStack() as pctx:
        prj = pctx.enter_context(tc.tile_pool(name="prjps", bufs=3, space="PSUM"))
        for cb in range(NCB):
            cslc = slice(cb * P, (cb + 1) * P)
            for sc in range(2):
                slc = slice(sc * 512, (sc + 1) * 512)
                ps_q = prj.tile([P, 512], FP32, tag="ps", name="ps_q")
                for db in range(ND):
                    nc.tensor.matmul(
                        ps_q[:],
                        lhsT=wq[db][:, cslc],
                        rhs=xq[db][:, slc],
                        start=(db == 0),
                        stop=False,
                    )
                nc.tensor.matmul(
                    ps_q[:],
                    lhsT=bqs[0:1, cslc],
                    rhs=ones[0:1, slc],
                    start=False,
                    stop=True,
                )
                nc.scalar.copy(QT[cb][:, slc], ps_q[:])

                ps_k = prj.tile([P, 512], FP32, tag="ps", name="ps_k")
                for db in range(ND):
                    nc.tensor.matmul(
                        ps_k[:],
                        lhsT=wk[db][:, cslc],
                        rhs=xk[db][:, slc],
                        start=(db == 0),
                        stop=False,
                    )
                nc.tensor.matmul(
                    ps_k[:],
                    lhsT=bks[0:1, cslc],
                    rhs=ones[0:1, slc],
                    start=False,
                    stop=True,
                )
                nc.scalar.copy(KT[cb][:, slc], ps_k[:])

        for sb in range(NS):
            ps_v = prj.tile([P, 512], FP32, tag="ps", name="ps_v")
            for db in range(ND):
                nc.tensor.matmul(
                    ps_v[:],
                    lhsT=xv[db][:, sb * P : (sb + 1) * P],
                    rhs=wv[db][:],
                    start=(db == 0),
                    stop=False,
                )
            nc.tensor.matmul(
                ps_v[:],
                lhsT=ones[0:1, sb * P : (sb + 1) * P],
                rhs=bvs[0:1, :],
                start=False,
                stop=True,
            )
            nc.scalar.copy(V[sb][:], ps_v[:])

    # ---- attention, one head-pair (one QT/KT tile) at a time ----
    with ExitStack() as actx:
        spool = actx.enter_context(tc.tile_pool(name="spsum", bufs=3, space="PSUM"))
        opool = actx.enter_context(tc.tile_pool(name="opsum", bufs=1, space="PSUM"))
        ppool = actx.enter_context(tc.tile_pool(name="pout", bufs=4))
        pbpool = actx.enter_context(tc.tile_pool(name="pb", bufs=4))
        etpool = actx.enter_context(tc.tile_pool(name="et", bufs=2))

        for h in range(HPC):
            hp = h // 2
            base = (h % 2) * DH
            # ET[kb][k, q] tiles filled by transposing the normalized bf16
            # attention rows; consumed by the P@V matmuls.
            ets = [
                etpool.tile([P, S], BF16, tag=f"et{kb}", name=f"et{kb}")
                for kb in range(NS)
            ]
            # phase A: S[q,k] = Q K^T; P = exp(S/8)/rowsum -> HBM + ET tiles
            for qb in range(NS):
                idx = h * NS + qb
                s_ps = spool.tile([P, S], FP32, tag="s", name="s_ps")
                for jc in range(2):
                    slc = slice(jc * 512, (jc + 1) * 512)
                    nc.tensor.matmul(
                        s_ps[:, slc],
                        lhsT=QT[hp][base : base + DH, qb * P : (qb + 1) * P],
                        rhs=KT[hp][base : base + DH, slc],
                        start=True,
                        stop=True,
                    )
                p_t = ppool.tile([P, S], FP32, tag="p", name="p_t")
                nc.scalar.activation(
                    p_t[:],
                    s_ps[:],
                    AF.Exp,
                    scale=SCALE,
                    accum_out=r_all[:, idx : idx + 1],
                )
                nc.vector.reciprocal(
                    rinv_all[:, idx : idx + 1], r_all[:, idx : idx + 1]
                )
                pb = pbpool.tile([P, S], BF16, tag="pb", name="pb")
                nc.vector.tensor_scalar_mul(pb[:], p_t[:], rinv_all[:, idx : idx + 1])
                nc.vector.tensor_scalar_mul(p_t[:], p_t[:], rinv_all[:, idx : idx + 1])
                dma(attn_d.ap()[h, qb * P : (qb + 1) * P, :], p_t[:])
                for kb in range(NS):
                    tdma(
                        ets[kb][:, qb * P : (qb + 1) * P],
                        pb[:, kb * P : (kb + 1) * P],
                    )

            # phase B: OT[c,q] = V^T @ P^T (already normalized)
            ot_ps = opool.tile([DH, S], FP32, tag="ot", name="ot_ps")
            for kb in range(NS):
                for jc in range(2):
                    slc = slice(jc * 512, (jc + 1) * 512)
                    nc.tensor.matmul(
                        ot_ps[:, slc],
                        lhsT=V[kb][:, h * DH : (h + 1) * DH],
                        rhs=ets[kb][:, slc],
                        start=(kb == 0),
                        stop=(kb == NS - 1),
                    )
            nc.vector.tensor_copy(OT[h][:], ot_ps[:])

    # ---- output projection: partial[q, o] = OT^T @ woT ----
    with ExitStack() as octx:
        opj = octx.enter_context(tc.tile_pool(name="opjps", bufs=2, space="PSUM"))
        osb = octx.enter_context(tc.tile_pool(name="osb", bufs=3))
        for qb in range(NS):
            for oc in range(2):
                slc = slice(oc * 512, (oc + 1) * 512)
                ps_o = opj.tile([P, 512], FP32, tag="po", name="ps_o")
                for h in range(HPC):
                    nc.tensor.matmul(
                        ps_o[:],
                        lhsT=OT[h][:, qb * P : (qb + 1) * P],
                        rhs=wo[h][:, slc],
                        start=(h == 0),
                        stop=(h == HPC - 1),
                    )
                o_sb = osb.tile([P, 512], FP32, tag="o", name="o_sb")
                nc.scalar.copy(o_sb[:], ps_o[:])
                dma(part_d.ap()[qb * P : (qb + 1) * P, slc], o_sb[:])


def _get_nc():
    if "nc" not in _CACHE:
        _CACHE["nc"] = _build_nc()
    return _CACHE["nc"]


def _prep_in_maps(q, k, v, Wq, bq, Wk, bk, Wv, bv, Wo):
    bf16 = ml_dtypes.bfloat16
    in_maps = []
    for c in range(8):
        b, g = divmod(c, 2)
        cs = slice(g * C, (g + 1) * C)
        in_maps.append(
            {
                "xqT": q[b].T.astype(bf16),
                "xkT": k[b].T.astype(bf16),
                "xvT": v[b].T.astype(bf16),
                "wqT": Wq[cs, :].T.astype(bf16),
                "wkT": Wk[cs, :].T.astype(bf16),
                "wvT": Wv[cs, :].T.astype(bf16),
                "bq": bq[cs].reshape(1, C).astype(bf16),
                "bk": bk[cs].reshape(1, C).astype(bf16),
                "bv": bv[cs].reshape(1, C).astype(bf16),
                "woT": Wo[:, cs].T.astype(bf16),
            }
        )
    return in_maps


def kernel(query, key_, value, Wq, bq, Wk, bk, Wv, bv, Wo, bo, _trace=False):
    from concourse.bass_utils import run_bass_kernel_spmd

    q = np.asarray(query, np.float32)
    k = np.asarray(key_, np.float32)
    v = np.asarray(value, np.float32)
    Wq = np.asarray(Wq, np.float32)
    bq = np.asarray(bq, np.float32)
    Wk = np.asarray(Wk, np.float32)
    bk = np.asarray(bk, np.float32)
    Wv = np.asarray(Wv, np.float32)
    bv = np.asarray(bv, np.float32)
    Wo = np.asarray(Wo, np.float32)
    bo = np.asarray(bo, np.float32)

    nc = _get_nc()
    in_maps = _prep_in_maps(q, k, v, Wq, bq, Wk, bk, Wv, bv, Wo)
    res = run_bass_kernel_spmd(nc, in_maps, list(range(8)), trace=_trace)
    _CACHE["last_result"] = res

    out = np.empty((B, S, D), np.float32)
    attention = np.empty((B, H, S, S), np.float32)
    for c in range(8):
        b, g = divmod(c, 2)
        attention[b, g * HPC : (g + 1) * HPC] = res.results[c]["attn"]
    for b in range(B):
        out[b] = (
            res.results[2 * b]["partial"]
            + res.results[2 * b + 1]["partial"]
            + bo[None, :]
        )
    return out, attention


# revision 25
# speedup vs baseline: 2.2353x; 2.2353x over previous
"""Trainium2 Bass kernel for 16-head MHA (B=4, S=1024, D=1024).

Returns (out, attention) matching the reference:
    Q/K/V = x @ W.T + b  (per-head split), scores = QK^T/sqrt(64),
    attention = softmax(scores), out = (attention @ V concat) @ Wo.T + bo.

Sharding: 8 cores = 4 batches x 2 head-groups (8 heads each).
Each core computes its batch's projections restricted to its 512
head-dims, full attention for its 8 heads, and a partial output
projection (contraction over its 512 head-dims). Host sums the two
partials per batch and adds bo.

Per-core layout (all matmul operands bf16, accumulation fp32):
  - inputs arrive transposed: xT[d, s] so projections need no on-chip
    transposes: QT/KT[c, s] (head-dim on partitions), V[s, c].
  - scores both ways: S[q, k] for the softmax/attention output (row
    softmax along free dim, exp row-sums via activation accum_out) and
    ST[k, q] for the P@V matmul (contraction needs k on partitions).
  - OT[c, q] = V^T @ exp(ST) accumulated over k blocks, normalized by
    broadcast 1/rowsum, then the output projection contracts c blocks.
"""

from contextlib import ExitStack

import ml_dtypes
import numpy as np

B = 4
S = 1024
D = 1024
H = 16
DH = 64
P = 128
C = 512  # head dims per core
HPC = 8  # heads per core
ND = D // P
NS = S // P
NCB = C // P
SCALE = 0.125  # 1 / sqrt(DH)

_CACHE = {}

# walrus in this environment rejects sequencer CTRL instructions (Drain/NoOp)
# carrying more than a couple of sem waits ("Too many sync wait commands").
# TileContext's tail drain waits on every active proc sem, so split it into a
# chain of drains with at most _MAX_CTRL_WAITS waits each.
_MAX_CTRL_WAITS = 1


def _patch_tail_drain():
    import concourse.mybir as mybir
    import concourse.tile as tile
    from concourse.vector_clock import ScopedClock

    if getattr(tile.TileContext._drain_and_barrier, "_split_waits", False):
        return

    def _drain_and_barrier(self, tick_clock, wait_clock):
        nc = self.nc
        drain_inst = nc.sync.drain()
        wait_clock.add_sem_waits(
            drain_inst.ins, ScopedClock({None: tick_clock.global_clock})
        )
        si = drain_inst.ins.sync_info
        waits = list(si.on_wait) if si is not None else []
        if len(waits) > _MAX_CTRL_WAITS:
            drain_inst.ins.sync_info = mybir.SyncInfo(
                on_wait=waits[:_MAX_CTRL_WAITS],
                on_update=list(si.on_update),
            )
            for i in range(_MAX_CTRL_WAITS, len(waits), _MAX_CTRL_WAITS):
                extra = nc.sync.drain()
                extra.ins.sync_info = mybir.SyncInfo(
                    on_wait=waits[i : i + _MAX_CTRL_WAITS], on_update=[]
                )
        nc.all_engine_barrier()
        assert self.sems is not None
        popped = nc._tile_sem_poison_stack.pop()
        assert popped is self._sem_poison
        nc.clear_and_free_semaphores(list(self.sems.allocated().values()))
        nc.all_engine_barrier()

    _drain_and_barrier._split_waits = True
    tile.TileContext._drain_and_barrier = _drain_and_barrier


def _split_sync_waits(nc, mybir, max_waits=1):
    """walrus here allows only one sync-wait command per instruction; hoist
    extra waits onto same-engine NoOps inserted just before the instruction
    (engine streams execute in block order, so this is equivalent)."""
    for fn in nc.m.functions:
        for blk in fn.blocks:
            out = []
            changed = False
            for inst in blk.instructions:
                si = inst.sync_info
                waits = list(si.on_wait) if si is not None else []
                if len(waits) > max_waits:
                    changed = True
                    for j, w in enumerate(waits[max_waits:]):
                        nop = mybir.InstNoOp(
                            name=f"{inst.name}-w{j}", ins=[], outs=[]
                        )
                        nop.engine = inst.engine
                        nop.sync_info = mybir.SyncInfo(on_wait=[w], on_update=[])
                        nc.register_instruction(nop, overwrite=True)
                        out.append(nop)
                    inst.sync_info = mybir.SyncInfo(
                        on_wait=waits[:max_waits], on_update=list(si.on_update)
                    )
                out.append(inst)
            if changed:
                blk.instructions = out


def _build_nc():
    import concourse.bass as bass
    import concourse.mybir as mybir
    import concourse.tile as tile

    _patch_tail_drain()

    FP32 = mybir.dt.float32
    BF16 = mybir.dt.bfloat16
    AF = mybir.ActivationFunctionType

    nc = bass.Bass("TRN2", target_bir_lowering=False, debug=False)

    xqT_d = nc.dram_tensor("xqT", [D, S], BF16, kind="ExternalInput")
    xkT_d = nc.dram_tensor("xkT", [D, S], BF16, kind="ExternalInput")
    xvT_d = nc.dram_tensor("xvT", [D, S], BF16, kind="ExternalInput")
    wqT_d = nc.dram_tensor("wqT", [D, C], BF16, kind="ExternalInput")
    wkT_d = nc.dram_tensor("wkT", [D, C], BF16, kind="ExternalInput")
    wvT_d = nc.dram_tensor("wvT", [D, C], BF16, kind="ExternalInput")
    bq_d = nc.dram_tensor("bq", [1, C], BF16, kind="ExternalInput")
    bk_d = nc.dram_tensor("bk", [1, C], BF16, kind="ExternalInput")
    bv_d = nc.dram_tensor("bv", [1, C], BF16, kind="ExternalInput")
    woT_d = nc.dram_tensor("woT", [C, D], BF16, kind="ExternalInput")
    attn_d = nc.dram_tensor("attn", [HPC, S, S], FP32, kind="ExternalOutput")
    part_d = nc.dram_tensor("partial", [S, D], FP32, kind="ExternalOutput")

    with tile.TileContext(nc) as tc:
        with ExitStack() as ctx:
            _body(
                ctx,
                tc,
                nc,
                FP32,
                BF16,
                AF,
                xqT_d,
                xkT_d,
                xvT_d,
                wqT_d,
                wkT_d,
                wvT_d,
                bq_d,
                bk_d,
                bv_d,
                woT_d,
                attn_d,
                part_d,
            )
    _split_sync_waits(nc, mybir)
    return nc


def _body(
    ctx,
    tc,
    nc,
    FP32,
    BF16,
    AF,
    xqT_d,
    xkT_d,
    xvT_d,
    wqT_d,
    wkT_d,
    wvT_d,
    bq_d,
    bk_d,
    bv_d,
    woT_d,
    attn_d,
    part_d,
):
    persist = ctx.enter_context(tc.tile_pool(name="persist", bufs=1))

    def ptile(name, shape, dt):
        return persist.tile(shape, dt, name=name)

    wo = [ptile(f"wo{i}", [DH, D], BF16) for i in range(HPC)]
    bqs = ptile("bqs", [1, C], BF16)
    bks = ptile("bks", [1, C], BF16)
    bvs = ptile("bvs", [1, C], BF16)
    ones = ptile("ones", [1, S], BF16)
    QT = [ptile(f"qt{i}", [P, S], BF16) for i in range(NCB)]
    KT = [ptile(f"kt{i}", [P, S], BF16) for i in range(NCB)]
    V = [ptile(f"v{i}", [P, C], BF16) for i in range(NS)]
    OT = [ptile(f"otc{i}", [DH, S], BF16) for i in range(HPC)]
    r_all = ptile("r_all", [P, HPC * NS], FP32)
    rinv_all = ptile("rinv_all", [P, HPC * NS], FP32)

    dma = nc.sync.dma_start
    tdma = nc.scalar.dma_start_transpose

    for i in range(HPC):
        dma(wo[i][:], woT_d.ap()[i * DH : (i + 1) * DH, :])
    dma(bqs[:], bq_d.ap()[:, :])
    dma(bks[:], bk_d.ap()[:, :])
    dma(bvs[:], bv_d.ap()[:, :])
    nc.vector.memset(ones[:], 1.0)

    # ---- projections: QT[c,s], KT[c,s] (transposed), V[s,c] (natural) ----
    with ExitStack() as pctx:
        xwpool = pctx.enter_context(tc.tile_pool(name="xw", bufs=1))
        xq = [xwpool.tile([P, S], BF16, name=f"xq{i}") for i in range(ND)]
        xk = [xwpool.tile([P, S], BF16, name=f"xk{i}") for i in range(ND)]
        xv = [xwpool.tile([P, S], BF16, name=f"xv{i}") for i in range(ND)]
        wq = [xwpool.tile([P, C], BF16, name=f"wq{i}") for i in range(ND)]
        wk = [xwpool.tile([P, C], BF16, name=f"wk{i}") for i in range(ND)]
        wv = [xwpool.tile([P, C], BF16, name=f"wv{i}") for i in range(ND)]
        for i in range(ND):
            dma(xq[i][:], xqT_d.ap()[i * P : (i + 1) * P, :])
            dma(xk[i][:], xkT_d.ap()[i * P : (i + 1) * P, :])
            dma(xv[i][:], xvT_d.ap()[i * P : (i + 1) * P, :])
            dma(wq[i][:], wqT_d.ap()[i * P : (i + 1) * P, :])
            dma(wk[i][:], wkT_d.ap()[i * P : (i + 1) * P, :])
            dma(wv[i][:], wvT_d.ap()[i * P : (i + 1) * P, :])
        prj = pctx.enter_context(tc.tile_pool(name="prjps", bufs=3, space="PSUM"))
        for cb in range(NCB):
            cslc = slice(cb * P, (cb + 1) * P)
            for sc in range(2):
                slc = slice(sc * 512, (sc + 1) * 512)
                ps_q = prj.tile([P, 512], FP32, tag="ps", name="ps_q")
                for db in range(ND):
                    nc.tensor.matmul(
                        ps_q[:],
                        lhsT=wq[db][:, cslc],
                        rhs=xq[db][:, slc],
                        start=(db == 0),
                        stop=False,
                    )
                nc.tensor.matmul(
                    ps_q[:],
                    lhsT=bqs[0:1, cslc],
                    rhs=ones[0:1, slc],
                    start=False,
                    stop=True,
                )
                nc.scalar.copy(QT[cb][:, slc], ps_q[:])

                ps_k = prj.tile([P, 512], FP32, tag="ps", name="ps_k")
                for db in range(ND):
                    nc.tensor.matmul(
                        ps_k[:],
                        lhsT=wk[db][:, cslc],
                        rhs=xk[db][:, slc],
                        start=(db == 0),
                        stop=False,
                    )
                nc.tensor.matmul(
                    ps_k[:],
                    lhsT=bks[0:1, cslc],
                    rhs=ones[0:1, slc],
                    start=False,
                    stop=True,
                )
                nc.scalar.copy(KT[cb][:, slc], ps_k[:])

        for sb in range(NS):
            ps_v = prj.tile([P, 512], FP32, tag="ps", name="ps_v")
            for db in range(ND):
                nc.tensor.matmul(
                    ps_v[:],
                    lhsT=xv[db][:, sb * P : (sb + 1) * P],
                    rhs=wv[db][:],
                    start=(db == 0),
                    stop=False,
                )
            nc.tensor.matmul(
                ps_v[:],
                lhsT=ones[0:1, sb * P : (sb + 1) * P],
                rhs=bvs[0:1, :],
                start=False,
                stop=True,
            )
            nc.scalar.copy(V[sb][:], ps_v[:])

    # ---- attention, one head-pair (one QT/KT tile) at a time ----
    with ExitStack() as actx:
        spool = actx.enter_context(tc.tile_pool(name="spsum", bufs=3, space="PSUM"))
        opool = actx.enter_context(tc.tile_pool(name="opsum", bufs=1, space="PSUM"))
        ppool = actx.enter_context(tc.tile_pool(name="pout", bufs=4))
        pbpool = actx.enter_context(tc.tile_pool(name="pb", bufs=4))
        etpool = actx.enter_context(tc.tile_pool(name="et", bufs=2))

        for h in range(HPC):
            hp = h // 2
            base = (h % 2) * DH
            # ET[k_lo, kb, q] filled by transposing the normalized bf16
            # attention rows (out[p, a, b] = in[b, a*128+p]); consumed by
            # the P@V matmuls as [128, 512] slices et[:, kb, chunk].
            et = etpool.tile([P, NS, S], BF16, tag="et", name="et")
            # phase A: S[q,k] = Q K^T; P = exp(S/8)/rowsum -> HBM + ET tiles
            for qb in range(NS):
                idx = h * NS + qb
                s_ps = spool.tile([P, S], FP32, tag="s", name="s_ps")
                for jc in range(2):
                    slc = slice(jc * 512, (jc + 1) * 512)
                    nc.tensor.matmul(
                        s_ps[:, slc],
                        lhsT=QT[hp][base : base + DH, qb * P : (qb + 1) * P],
                        rhs=KT[hp][base : base + DH, slc],
                        start=True,
                        stop=True,
                    )
                p_t = ppool.tile([P, S], FP32, tag="p", name="p_t")
                nc.scalar.activation(
                    p_t[:],
                    s_ps[:],
                    AF.Exp,
                    scale=SCALE,
                    accum_out=r_all[:, idx : idx + 1],
                )
                nc.vector.reciprocal(
                    rinv_all[:, idx : idx + 1], r_all[:, idx : idx + 1]
                )
                pb = pbpool.tile([P, S], BF16, tag="pb", name="pb")
                nc.vector.tensor_scalar_mul(pb[:], p_t[:], rinv_all[:, idx : idx + 1])
                nc.vector.tensor_scalar_mul(p_t[:], p_t[:], rinv_all[:, idx : idx + 1])
                dma(attn_d.ap()[h, qb * P : (qb + 1) * P, :], p_t[:])
                tdma(et[:, :, qb * P : (qb + 1) * P], pb[:])

            # phase B: OT[c,q] = V^T @ P^T (already normalized)
            ot_ps = opool.tile([DH, S], FP32, tag="ot", name="ot_ps")
            for kb in range(NS):
                for jc in range(2):
                    slc = slice(jc * 512, (jc + 1) * 512)
                    nc.tensor.matmul(
                        ot_ps[:, slc],
                        lhsT=V[kb][:, h * DH : (h + 1) * DH],
                        rhs=et[:, kb, slc],
                        start=(kb == 0),
                        stop=(kb == NS - 1),
                    )
            nc.vector.tensor_copy(OT[h][:], ot_ps[:])

    # ---- output projection: partial[q, o] = OT^T @ woT ----
    with ExitStack() as octx:
        opj = octx.enter_context(tc.tile_pool(name="opjps", bufs=2, space="PSUM"))
        osb = octx.enter_context(tc.tile_pool(name="osb", bufs=3))
        for qb in range(NS):
            for oc in range(2):
                slc = slice(oc * 512, (oc + 1) * 512)
                ps_o = opj.tile([P, 512], FP32, tag="po", name="ps_o")
                for h in range(HPC):
                    nc.tensor.matmul(
                        ps_o[:],
                        lhsT=OT[h][:, qb * P : (qb + 1) * P],
                        rhs=wo[h][:, slc],
                        start=(h == 0),
                        stop=(h == HPC - 1),
                    )
                o_sb = osb.tile([P, 512], FP32, tag="o", name="o_sb")
                nc.scalar.copy(o_sb[:], ps_o[:])
                dma(part_d.ap()[qb * P : (qb + 1) * P, slc], o_sb[:])


def _get_nc():
    if "nc" not in _CACHE:
        _CACHE["nc"] = _build_nc()
    return _CACHE["nc"]


def _prep_in_maps(q, k, v, Wq, bq, Wk, bk, Wv, bv, Wo):
    bf16 = ml_dtypes.bfloat16
    in_maps = []
    for c in range(8):
        b, g = divmod(c, 2)
        cs = slice(g * C, (g + 1) * C)
        in_maps.append(
            {
                "xqT": q[b].T.astype(bf16),
                "xkT": k[b].T.astype(bf16),
                "xvT": v[b].T.astype(bf16),
                "wqT": Wq[cs, :].T.astype(bf16),
                "wkT": Wk[cs, :].T.astype(bf16),
                "wvT": Wv[cs, :].T.astype(bf16),
                "bq": bq[cs].reshape(1, C).astype(bf16),
                "bk": bk[cs].reshape(1, C).astype(bf16),
                "bv": bv[cs].reshape(1, C).astype(bf16),
                "woT": Wo[:, cs].T.astype(bf16),
            }
        )
    return in_maps


def kernel(query, key_, value, Wq, bq, Wk, bk, Wv, bv, Wo, bo, _trace=False):
    from concourse.bass_utils import run_bass_kernel_spmd

    q = np.asarray(query, np.float32)
    k = np.asarray(key_, np.float32)
    v = np.asarray(value, np.float32)
    Wq = np.asarray(Wq, np.float32)
    bq = np.asarray(bq, np.float32)
    Wk = np.asarray(Wk, np.float32)
    bk = np.asarray(bk, np.float32)
    Wv = np.asarray(Wv, np.float32)
    bv = np.asarray(bv, np.float32)
    Wo = np.asarray(Wo, np.float32)
    bo = np.asarray(bo, np.float32)

    nc = _get_nc()
    in_maps = _prep_in_maps(q, k, v, Wq, bq, Wk, bk, Wv, bv, Wo)
    res = run_bass_kernel_spmd(nc, in_maps, list(range(8)), trace=_trace)
    _CACHE["last_result"] = res

    out = np.empty((B, S, D), np.float32)
    attention = np.empty((B, H, S, S), np.float32)
    for c in range(8):
        b, g = divmod(c, 2)
        attention[b, g * HPC : (g + 1) * HPC] = res.results[c]["attn"]
    for b in range(B):
        out[b] = (
            res.results[2 * b]["partial"]
            + res.results[2 * b + 1]["partial"]
            + bo[None, :]
        )
    return out, attention


# revision 28
# speedup vs baseline: 2.7107x; 1.2127x over previous
"""Trainium2 Bass kernel for 16-head MHA (B=4, S=1024, D=1024).

Returns (out, attention) matching the reference:
    Q/K/V = x @ W.T + b  (per-head split), scores = QK^T/sqrt(64),
    attention = softmax(scores), out = (attention @ V concat) @ Wo.T + bo.

Sharding: 8 cores = 4 batches x 2 head-groups (8 heads each).
Each core computes its batch's projections restricted to its 512
head-dims, full attention for its 8 heads, and a partial output
projection (contraction over its 512 head-dims). Host sums the two
partials per batch and adds bo.

Per-core layout (all matmul operands bf16, accumulation fp32):
  - inputs arrive transposed: xT[d, s] so projections need no on-chip
    transposes: QT/KT[c, s] (head-dim on partitions), V[s, c].
  - scores both ways: S[q, k] for the softmax/attention output (row
    softmax along free dim, exp row-sums via activation accum_out) and
    ST[k, q] for the P@V matmul (contraction needs k on partitions).
  - OT[c, q] = V^T @ exp(ST) accumulated over k blocks, normalized by
    broadcast 1/rowsum, then the output projection contracts c blocks.
"""

from contextlib import ExitStack

import ml_dtypes
import numpy as np

B = 4
S = 1024
D = 1024
H = 16
DH = 64
P = 128
C = 512  # head dims per core
HPC = 8  # heads per core
ND = D // P
NS = S // P
NCB = C // P
SCALE = 0.125  # 1 / sqrt(DH)

_CACHE = {}

# walrus in this environment rejects sequencer CTRL instructions (Drain/NoOp)
# carrying more than a couple of sem waits ("Too many sync wait commands").
# TileContext's tail drain waits on every active proc sem, so split it into a
# chain of drains with at most _MAX_CTRL_WAITS waits each.
_MAX_CTRL_WAITS = 1


def _patch_tail_drain():
    import concourse.mybir as mybir
    import concourse.tile as tile
    from concourse.vector_clock import ScopedClock

    if getattr(tile.TileContext._drain_and_barrier, "_split_waits", False):
        return

    def _drain_and_barrier(self, tick_clock, wait_clock):
        nc = self.nc
        drain_inst = nc.sync.drain()
        wait_clock.add_sem_waits(
            drain_inst.ins, ScopedClock({None: tick_clock.global_clock})
        )
        si = drain_inst.ins.sync_info
        waits = list(si.on_wait) if si is not None else []
        if len(waits) > _MAX_CTRL_WAITS:
            drain_inst.ins.sync_info = mybir.SyncInfo(
                on_wait=waits[:_MAX_CTRL_WAITS],
                on_update=list(si.on_update),
            )
            for i in range(_MAX_CTRL_WAITS, len(waits), _MAX_CTRL_WAITS):
                extra = nc.sync.drain()
                extra.ins.sync_info = mybir.SyncInfo(
                    on_wait=waits[i : i + _MAX_CTRL_WAITS], on_update=[]
                )
        nc.all_engine_barrier()
        assert self.sems is not None
        popped = nc._tile_sem_poison_stack.pop()
        assert popped is self._sem_poison
        nc.clear_and_free_semaphores(list(self.sems.allocated().values()))
        nc.all_engine_barrier()

    _drain_and_barrier._split_waits = True
    tile.TileContext._drain_and_barrier = _drain_and_barrier


def _split_sync_waits(nc, mybir, max_waits=1):
    """walrus here allows only one sync-wait command per instruction; hoist
    extra waits onto same-engine NoOps inserted just before the instruction
    (engine streams execute in block order, so this is equivalent)."""
    for fn in nc.m.functions:
        for blk in fn.blocks:
            out = []
            changed = False
            for inst in blk.instructions:
                si = inst.sync_info
                waits = list(si.on_wait) if si is not None else []
                if len(waits) > max_waits:
                    changed = True
                    for j, w in enumerate(waits[max_waits:]):
                        nop = mybir.InstNoOp(
                            name=f"{inst.name}-w{j}", ins=[], outs=[]
                        )
                        nop.engine = inst.engine
                        nop.sync_info = mybir.SyncInfo(on_wait=[w], on_update=[])
                        nc.register_instruction(nop, overwrite=True)
                        out.append(nop)
                    inst.sync_info = mybir.SyncInfo(
                        on_wait=waits[:max_waits], on_update=list(si.on_update)
                    )
                out.append(inst)
            if changed:
                blk.instructions = out


def _build_nc():
    import concourse.bass as bass
    import concourse.mybir as mybir
    import concourse.tile as tile

    _patch_tail_drain()

    FP32 = mybir.dt.float32
    BF16 = mybir.dt.bfloat16
    AF = mybir.ActivationFunctionType

    nc = bass.Bass("TRN2", target_bir_lowering=False, debug=False)

    xqT_d = nc.dram_tensor("xqT", [D, S], BF16, kind="ExternalInput")
    xkT_d = nc.dram_tensor("xkT", [D, S], BF16, kind="ExternalInput")
    xvT_d = nc.dram_tensor("xvT", [D, S], BF16, kind="ExternalInput")
    wqT_d = nc.dram_tensor("wqT", [D, C], BF16, kind="ExternalInput")
    wkT_d = nc.dram_tensor("wkT", [D, C], BF16, kind="ExternalInput")
    wvT_d = nc.dram_tensor("wvT", [D, C], BF16, kind="ExternalInput")
    bq_d = nc.dram_tensor("bq", [1, C], BF16, kind="ExternalInput")
    bk_d = nc.dram_tensor("bk", [1, C], BF16, kind="ExternalInput")
    bv_d = nc.dram_tensor("bv", [1, C], BF16, kind="ExternalInput")
    woT_d = nc.dram_tensor("woT", [C, D], BF16, kind="ExternalInput")
    attn_d = nc.dram_tensor("attn", [HPC, S, S], FP32, kind="ExternalOutput")
    part_d = nc.dram_tensor("partial", [S, D], FP32, kind="ExternalOutput")

    with tile.TileContext(nc) as tc:
        with ExitStack() as ctx:
            _body(
                ctx,
                tc,
                nc,
                FP32,
                BF16,
                AF,
                xqT_d,
                xkT_d,
                xvT_d,
                wqT_d,
                wkT_d,
                wvT_d,
                bq_d,
                bk_d,
                bv_d,
                woT_d,
                attn_d,
                part_d,
            )
    _split_sync_waits(nc, mybir)
    return nc


def _body(
    ctx,
    tc,
    nc,
    FP32,
    BF16,
    AF,
    xqT_d,
    xkT_d,
    xvT_d,
    wqT_d,
    wkT_d,
    wvT_d,
    bq_d,
    bk_d,
    bv_d,
    woT_d,
    attn_d,
    part_d,
):
    persist = ctx.enter_context(tc.tile_pool(name="persist", bufs=1))

    def ptile(name, shape, dt):
        return persist.tile(shape, dt, name=name)

    wo = [ptile(f"wo{i}", [P, D], BF16) for i in range(NCB)]
    bqs = ptile("bqs", [1, C], BF16)
    bks = ptile("bks", [1, C], BF16)
    bvs = ptile("bvs", [1, C], BF16)
    ones = ptile("ones", [1, S], BF16)
    onesf = ptile("onesf", [1, P], FP32)
    QT = [ptile(f"qt{i}", [P, S], BF16) for i in range(NCB)]
    KT = [ptile(f"kt{i}", [P, S], BF16) for i in range(NCB)]
    V = [ptile(f"v{i}", [P, C], BF16) for i in range(NS)]
    OT = [ptile(f"otc{i}", [P, S], BF16) for i in range(NCB)]
    r_all = ptile("r_all", [P, HPC * NS], FP32)
    rinv_all = ptile("rinv_all", [P, HPC * NS], FP32)

    dma = nc.sync.dma_start

    for i in range(NCB):
        dma(wo[i][:], woT_d.ap()[i * P : (i + 1) * P, :])
    dma(bqs[:], bq_d.ap()[:, :])
    dma(bks[:], bk_d.ap()[:, :])
    dma(bvs[:], bv_d.ap()[:, :])
    nc.vector.memset(ones[:], 1.0)
    nc.vector.memset(onesf[:], 1.0)

    # ---- projections: QT[c,s], KT[c,s] (transposed), V[s,c] (natural) ----
    with ExitStack() as pctx:
        xwpool = pctx.enter_context(tc.tile_pool(name="xw", bufs=1))
        xq = [xwpool.tile([P, S], BF16, name=f"xq{i}") for i in range(ND)]
        xk = [xwpool.tile([P, S], BF16, name=f"xk{i}") for i in range(ND)]
        xv = [xwpool.tile([P, S], BF16, name=f"xv{i}") for i in range(ND)]
        wq = [xwpool.tile([P, C], BF16, name=f"wq{i}") for i in range(ND)]
        wk = [xwpool.tile([P, C], BF16, name=f"wk{i}") for i in range(ND)]
        wv = [xwpool.tile([P, C], BF16, name=f"wv{i}") for i in range(ND)]
        for i in range(ND):
            dma(xq[i][:], xqT_d.ap()[i * P : (i + 1) * P, :])
            dma(xk[i][:], xkT_d.ap()[i * P : (i + 1) * P, :])
            dma(xv[i][:], xvT_d.ap()[i * P : (i + 1) * P, :])
            dma(wq[i][:], wqT_d.ap()[i * P : (i + 1) * P, :])
            dma(wk[i][:], wkT_d.ap()[i * P : (i + 1) * P, :])
            dma(wv[i][:], wvT_d.ap()[i * P : (i + 1) * P, :])
        prj = pctx.enter_context(tc.tile_pool(name="prjps", bufs=3, space="PSUM"))
        for cb in range(NCB):
            cslc = slice(cb * P, (cb + 1) * P)
            for sc in range(2):
                slc = slice(sc * 512, (sc + 1) * 512)
                ps_q = prj.tile([P, 512], FP32, tag="ps", name="ps_q")
                for db in range(ND):
                    nc.tensor.matmul(
                        ps_q[:],
                        lhsT=wq[db][:, cslc],
                        rhs=xq[db][:, slc],
                        start=(db == 0),
                        stop=False,
                    )
                nc.tensor.matmul(
                    ps_q[:],
                    lhsT=bqs[0:1, cslc],
                    rhs=ones[0:1, slc],
                    start=False,
                    stop=True,
                )
                nc.scalar.copy(QT[cb][:, slc], ps_q[:])

                ps_k = prj.tile([P, 512], FP32, tag="ps", name="ps_k")
                for db in range(ND):
                    nc.tensor.matmul(
                        ps_k[:],
                        lhsT=wk[db][:, cslc],
                        rhs=xk[db][:, slc],
                        start=(db == 0),
                        stop=False,
                    )
                nc.tensor.matmul(
                    ps_k[:],
                    lhsT=bks[0:1, cslc],
                    rhs=ones[0:1, slc],
                    start=False,
                    stop=True,
                )
                nc.scalar.copy(KT[cb][:, slc], ps_k[:])

        for sb in range(NS):
            ps_v = prj.tile([P, 512], FP32, tag="ps", name="ps_v")
            for db in range(ND):
                nc.tensor.matmul(
                    ps_v[:],
                    lhsT=xv[db][:, sb * P : (sb + 1) * P],
                    rhs=wv[db][:],
                    start=(db == 0),
                    stop=False,
                )
            nc.tensor.matmul(
                ps_v[:],
                lhsT=ones[0:1, sb * P : (sb + 1) * P],
                rhs=bvs[0:1, :],
                start=False,
                stop=True,
            )
            nc.scalar.copy(V[sb][:], ps_v[:])

    # ---- attention, one head-pair (one QT/KT tile) at a time ----
    with ExitStack() as actx:
        psA = actx.enter_context(tc.tile_pool(name="psA", bufs=4, space="PSUM"))
        ppool = actx.enter_context(tc.tile_pool(name="pout", bufs=4))
        epool = actx.enter_context(tc.tile_pool(name="et", bufs=3))
        rpool = actx.enter_context(tc.tile_pool(name="rr", bufs=2))

        for hp in range(NCB):
            # phase A: S[q,k] = Q K^T; even/odd heads issued adjacently so
            # their K=64 matmuls pack onto disjoint PE row groups.
            for qb in range(NS):
                s_e = psA.tile([P, S], FP32, tag="s", name="s_e")
                s_o = psA.tile([P, S], FP32, tag="s", name="s_o")
                for jc in range(2):
                    slc = slice(jc * 512, (jc + 1) * 512)
                    for par, s_ps in ((0, s_e), (1, s_o)):
                        base = par * DH
                        nc.tensor.matmul(
                            s_ps[:, slc],
                            lhsT=QT[hp][base : base + DH, qb * P : (qb + 1) * P],
                            rhs=KT[hp][base : base + DH, slc],
                            start=True,
                            stop=True,
                        )
                for par, s_ps in ((0, s_e), (1, s_o)):
                    idx = (2 * hp + par) * NS + qb
                    p_t = ppool.tile([P, S], FP32, tag="p", name="p_t")
                    nc.scalar.activation(
                        p_t[:],
                        s_ps[:],
                        AF.Exp,
                        scale=SCALE,
                        accum_out=r_all[:, idx : idx + 1],
                    )
                    nc.vector.reciprocal(
                        rinv_all[:, idx : idx + 1], r_all[:, idx : idx + 1]
                    )
                    nc.vector.tensor_scalar_mul(
                        p_t[:], p_t[:], rinv_all[:, idx : idx + 1]
                    )
                    dma(
                        attn_d.ap()[2 * hp + par, qb * P : (qb + 1) * P, :],
                        p_t[:],
                    )

            # 1/rowsum rows -> broadcast across partitions via PE outer
            # product (even rows 0:64, odd rows 64:128 of one tile).
            rrows = []
            for par in range(2):
                h = 2 * hp + par
                rrow = rpool.tile([1, S], FP32, tag=f"rrow{par}", name=f"rrow{par}")
                for qb in range(NS):
                    idx = h * NS + qb
                    nc.gpsimd.dma_start(
                        rrow[0:1, qb * P : (qb + 1) * P],
                        rinv_all[:, idx : idx + 1],
                    )
                rrows.append(rrow)
            rbc_ps = psA.tile([P, S], FP32, tag="s", name="rbc_ps")
            for par in range(2):
                kwargs = {} if par == 0 else {"tile_position": (0, DH)}
                for jc in range(2):
                    slc = slice(jc * 512, (jc + 1) * 512)
                    nc.tensor.matmul(
                        rbc_ps[par * DH : (par + 1) * DH, slc],
                        lhsT=onesf[0:1, 0:DH],
                        rhs=rrows[par][0:1, slc],
                        start=True,
                        stop=True,
                        **kwargs,
                    )
            rbc = rpool.tile([P, S], FP32, tag="rbc", name="rbc")
            nc.vector.tensor_copy(rbc[:], rbc_ps[:])

            # phase B: ST[k,q] = K Q^T (packed pairs), exp -> bf16, then
            # OT = V^T @ exp(ST): even head accumulates into rows 0:64 of
            # otE, odd head into rows 64:128 of otO (col group 64) so the
            # evict multiply stays lane-aligned.
            otE = psA.tile([P, S], FP32, tag="s", name="otE")
            otO = psA.tile([P, S], FP32, tag="s", name="otO")
            for kb in range(NS):
                st_e = psA.tile([P, S], FP32, tag="s", name="st_e")
                st_o = psA.tile([P, S], FP32, tag="s", name="st_o")
                for jc in range(2):
                    slc = slice(jc * 512, (jc + 1) * 512)
                    for par, st in ((0, st_e), (1, st_o)):
                        base = par * DH
                        nc.tensor.matmul(
                            st[:, slc],
                            lhsT=KT[hp][base : base + DH, kb * P : (kb + 1) * P],
                            rhs=QT[hp][base : base + DH, slc],
                            start=True,
                            stop=True,
                        )
                et_e = epool.tile([P, S], BF16, tag="ete", name="et_e")
                et_o = epool.tile([P, S], BF16, tag="eto", name="et_o")
                nc.scalar.activation(et_e[:], st_e[:], AF.Exp, scale=SCALE)
                nc.scalar.activation(et_o[:], st_o[:], AF.Exp, scale=SCALE)
                for jc in range(2):
                    slc = slice(jc * 512, (jc + 1) * 512)
                    for par, et in ((0, et_e), (1, et_o)):
                        h = 2 * hp + par
                        if par == 0:
                            out_ap = otE[0:DH, slc]
                            kwargs = {}
                        else:
                            out_ap = otO[DH:P, slc]
                            kwargs = {"tile_position": (0, DH)}
                        nc.tensor.matmul(
                            out_ap,
                            lhsT=V[kb][:, h * DH : (h + 1) * DH],
                            rhs=et[:, slc],
                            start=(kb == 0),
                            stop=(kb == NS - 1),
                            **kwargs,
                        )
            nc.vector.tensor_mul(OT[hp][0:DH, :], otE[0:DH, :], rbc[0:DH, :])
            nc.vector.tensor_mul(OT[hp][DH:P, :], otO[DH:P, :], rbc[DH:P, :])

    # ---- output projection: partial[q, o] = OT^T @ woT (K=128 blocks) ----
    with ExitStack() as octx:
        opj = octx.enter_context(tc.tile_pool(name="opjps", bufs=2, space="PSUM"))
        osb = octx.enter_context(tc.tile_pool(name="osb", bufs=3))
        for qb in range(NS):
            for oc in range(2):
                slc = slice(oc * 512, (oc + 1) * 512)
                ps_o = opj.tile([P, 512], FP32, tag="po", name="ps_o")
                for cb in range(NCB):
                    nc.tensor.matmul(
                        ps_o[:],
                        lhsT=OT[cb][:, qb * P : (qb + 1) * P],
                        rhs=wo[cb][:, slc],
                        start=(cb == 0),
                        stop=(cb == NCB - 1),
                    )
                o_sb = osb.tile([P, 512], FP32, tag="o", name="o_sb")
                nc.scalar.copy(o_sb[:], ps_o[:])
                dma(part_d.ap()[qb * P : (qb + 1) * P, slc], o_sb[:])


def _get_nc():
    if "nc" not in _CACHE:
        _CACHE["nc"] = _build_nc()
    return _CACHE["nc"]


def _prep_in_maps(q, k, v, Wq, bq, Wk, bk, Wv, bv, Wo):
    bf16 = ml_dtypes.bfloat16
    in_maps = []
    for c in range(8):
        b, g = divmod(c, 2)
        cs = slice(g * C, (g + 1) * C)
        in_maps.append(
            {
                "xqT": q[b].T.astype(bf16),
                "xkT": k[b].T.astype(bf16),
                "xvT": v[b].T.astype(bf16),
                "wqT": Wq[cs, :].T.astype(bf16),
                "wkT": Wk[cs, :].T.astype(bf16),
                "wvT": Wv[cs, :].T.astype(bf16),
                "bq": bq[cs].reshape(1, C).astype(bf16),
                "bk": bk[cs].reshape(1, C).astype(bf16),
                "bv": bv[cs].reshape(1, C).astype(bf16),
                "woT": Wo[:, cs].T.astype(bf16),
            }
        )
    return in_maps


def kernel(query, key_, value, Wq, bq, Wk, bk, Wv, bv, Wo, bo, _trace=False):
    from concourse.bass_utils import run_bass_kernel_spmd

    q = np.asarray(query, np.float32)
    k = np.asarray(key_, np.float32)
    v = np.asarray(value, np.float32)
    Wq = np.asarray(Wq, np.float32)
    bq = np.asarray(bq, np.float32)
    Wk = np.asarray(Wk, np.float32)
    bk = np.asarray(bk, np.float32)
    Wv = np.asarray(Wv, np.float32)
    bv = np.asarray(bv, np.float32)
    Wo = np.asarray(Wo, np.float32)
    bo = np.asarray(bo, np.float32)

    nc = _get_nc()
    in_maps = _prep_in_maps(q, k, v, Wq, bq, Wk, bk, Wv, bv, Wo)
    res = run_bass_kernel_spmd(nc, in_maps, list(range(8)), trace=_trace)
    _CACHE["last_result"] = res

    out = np.empty((B, S, D), np.float32)
    attention = np.empty((B, H, S, S), np.float32)
    for c in range(8):
        b, g = divmod(c, 2)
        attention[b, g * HPC : (g + 1) * HPC] = res.results[c]["attn"]
    for b in range(B):
        out[b] = (
            res.results[2 * b]["partial"]
            + res.results[2 * b + 1]["partial"]
            + bo[None, :]
        )
    return out, attention


# revision 29
# speedup vs baseline: 3.1359x; 1.1569x over previous
"""Trainium2 Bass kernel for 16-head MHA (B=4, S=1024, D=1024).

Returns (out, attention) matching the reference:
    Q/K/V = x @ W.T + b  (per-head split), scores = QK^T/sqrt(64),
    attention = softmax(scores), out = (attention @ V concat) @ Wo.T + bo.

Sharding: 8 cores = 4 batches x 2 head-groups (8 heads each).
Each core computes its batch's projections restricted to its 512
head-dims, full attention for its 8 heads, and a partial output
projection (contraction over its 512 head-dims). Host sums the two
partials per batch and adds bo.

Per-core layout (all matmul operands bf16, accumulation fp32):
  - inputs arrive transposed: xT[d, s] so projections need no on-chip
    transposes: QT/KT[c, s] (head-dim on partitions), V[s, c].
  - scores both ways: S[q, k] for the softmax/attention output (row
    softmax along free dim, exp row-sums via activation accum_out) and
    ST[k, q] for the P@V matmul (contraction needs k on partitions).
  - OT[c, q] = V^T @ exp(ST) accumulated over k blocks, normalized by
    broadcast 1/rowsum, then the output projection contracts c blocks.
"""

from contextlib import ExitStack

import ml_dtypes
import numpy as np

B = 4
S = 1024
D = 1024
H = 16
DH = 64
P = 128
C = 512  # head dims per core
HPC = 8  # heads per core
ND = D // P
NS = S // P
NCB = C // P
SCALE = 0.125  # 1 / sqrt(DH)

_CACHE = {}

# walrus in this environment rejects sequencer CTRL instructions (Drain/NoOp)
# carrying more than a couple of sem waits ("Too many sync wait commands").
# TileContext's tail drain waits on every active proc sem, so split it into a
# chain of drains with at most _MAX_CTRL_WAITS waits each.
_MAX_CTRL_WAITS = 1


def _patch_tail_drain():
    import concourse.mybir as mybir
    import concourse.tile as tile
    from concourse.vector_clock import ScopedClock

    if getattr(tile.TileContext._drain_and_barrier, "_split_waits", False):
        return

    def _drain_and_barrier(self, tick_clock, wait_clock):
        nc = self.nc
        drain_inst = nc.sync.drain()
        wait_clock.add_sem_waits(
            drain_inst.ins, ScopedClock({None: tick_clock.global_clock})
        )
        si = drain_inst.ins.sync_info
        waits = list(si.on_wait) if si is not None else []
        if len(waits) > _MAX_CTRL_WAITS:
            drain_inst.ins.sync_info = mybir.SyncInfo(
                on_wait=waits[:_MAX_CTRL_WAITS],
                on_update=list(si.on_update),
            )
            for i in range(_MAX_CTRL_WAITS, len(waits), _MAX_CTRL_WAITS):
                extra = nc.sync.drain()
                extra.ins.sync_info = mybir.SyncInfo(
                    on_wait=waits[i : i + _MAX_CTRL_WAITS], on_update=[]
                )
        nc.all_engine_barrier()
        assert self.sems is not None
        popped = nc._tile_sem_poison_stack.pop()
        assert popped is self._sem_poison
        nc.clear_and_free_semaphores(list(self.sems.allocated().values()))
        nc.all_engine_barrier()

    _drain_and_barrier._split_waits = True
    tile.TileContext._drain_and_barrier = _drain_and_barrier


def _split_sync_waits(nc, mybir, max_waits=1):
    """walrus here allows only one sync-wait command per instruction; hoist
    extra waits onto same-engine NoOps inserted just before the instruction
    (engine streams execute in block order, so this is equivalent)."""
    for fn in nc.m.functions:
        for blk in fn.blocks:
            out = []
            changed = False
            for inst in blk.instructions:
                si = inst.sync_info
                waits = list(si.on_wait) if si is not None else []
                if len(waits) > max_waits:
                    changed = True
                    for j, w in enumerate(waits[max_waits:]):
                        nop = mybir.InstNoOp(
                            name=f"{inst.name}-w{j}", ins=[], outs=[]
                        )
                        nop.engine = inst.engine
                        nop.sync_info = mybir.SyncInfo(on_wait=[w], on_update=[])
                        nc.register_instruction(nop, overwrite=True)
                        out.append(nop)
                    inst.sync_info = mybir.SyncInfo(
                        on_wait=waits[:max_waits], on_update=list(si.on_update)
                    )
                out.append(inst)
            if changed:
                blk.instructions = out


def _build_nc():
    import concourse.bass as bass
    import concourse.mybir as mybir
    import concourse.tile as tile

    _patch_tail_drain()

    FP32 = mybir.dt.float32
    BF16 = mybir.dt.bfloat16
    AF = mybir.ActivationFunctionType

    nc = bass.Bass("TRN2", target_bir_lowering=False, debug=False)

    xqT_d = nc.dram_tensor("xqT", [D, S], BF16, kind="ExternalInput")
    xkT_d = nc.dram_tensor("xkT", [D, S], BF16, kind="ExternalInput")
    xvT_d = nc.dram_tensor("xvT", [D, S], BF16, kind="ExternalInput")
    wqT_d = nc.dram_tensor("wqT", [D, C], BF16, kind="ExternalInput")
    wkT_d = nc.dram_tensor("wkT", [D, C], BF16, kind="ExternalInput")
    wvT_d = nc.dram_tensor("wvT", [D, C], BF16, kind="ExternalInput")
    bq_d = nc.dram_tensor("bq", [1, C], BF16, kind="ExternalInput")
    bk_d = nc.dram_tensor("bk", [1, C], BF16, kind="ExternalInput")
    bv_d = nc.dram_tensor("bv", [1, C], BF16, kind="ExternalInput")
    woT_d = nc.dram_tensor("woT", [C, D], BF16, kind="ExternalInput")
    attn_d = nc.dram_tensor("attn", [HPC, S, S], FP32, kind="ExternalOutput")
    part_d = nc.dram_tensor("partial", [S, D], FP32, kind="ExternalOutput")

    with tile.TileContext(nc) as tc:
        with ExitStack() as ctx:
            _body(
                ctx,
                tc,
                nc,
                FP32,
                BF16,
                AF,
                xqT_d,
                xkT_d,
                xvT_d,
                wqT_d,
                wkT_d,
                wvT_d,
                bq_d,
                bk_d,
                bv_d,
                woT_d,
                attn_d,
                part_d,
            )
    _split_sync_waits(nc, mybir)
    return nc


def _body(
    ctx,
    tc,
    nc,
    FP32,
    BF16,
    AF,
    xqT_d,
    xkT_d,
    xvT_d,
    wqT_d,
    wkT_d,
    wvT_d,
    bq_d,
    bk_d,
    bv_d,
    woT_d,
    attn_d,
    part_d,
):
    persist = ctx.enter_context(tc.tile_pool(name="persist", bufs=1))

    def ptile(name, shape, dt):
        return persist.tile(shape, dt, name=name)

    wo = [ptile(f"wo{i}", [P, D], BF16) for i in range(NCB)]
    bqs = ptile("bqs", [1, C], BF16)
    bks = ptile("bks", [1, C], BF16)
    bvs = ptile("bvs", [1, C], BF16)
    ones = ptile("ones", [1, S], BF16)
    onesf = ptile("onesf", [1, P], FP32)
    QT = [ptile(f"qt{i}", [P, S], BF16) for i in range(NCB)]
    KT = [ptile(f"kt{i}", [P, S], BF16) for i in range(NCB)]
    V = [ptile(f"v{i}", [P, C], BF16) for i in range(NS)]
    OT = [ptile(f"otc{i}", [P, S], BF16) for i in range(NCB)]
    r_all = ptile("r_all", [P, HPC * NS], FP32)
    rinv_all = ptile("rinv_all", [P, HPC * NS], FP32)

    dma = nc.sync.dma_start

    for i in range(NCB):
        dma(wo[i][:], woT_d.ap()[i * P : (i + 1) * P, :])
    dma(bqs[:], bq_d.ap()[:, :])
    dma(bks[:], bk_d.ap()[:, :])
    dma(bvs[:], bv_d.ap()[:, :])
    nc.vector.memset(ones[:], 1.0)
    nc.vector.memset(onesf[:], 1.0)

    # ---- projections: QT[c,s], KT[c,s] (transposed), V[s,c] (natural) ----
    with ExitStack() as pctx:
        xwpool = pctx.enter_context(tc.tile_pool(name="xw", bufs=1))
        xq = [xwpool.tile([P, S], BF16, name=f"xq{i}") for i in range(ND)]
        xk = [xwpool.tile([P, S], BF16, name=f"xk{i}") for i in range(ND)]
        xv = [xwpool.tile([P, S], BF16, name=f"xv{i}") for i in range(ND)]
        wq = [xwpool.tile([P, C], BF16, name=f"wq{i}") for i in range(ND)]
        wk = [xwpool.tile([P, C], BF16, name=f"wk{i}") for i in range(ND)]
        wv = [xwpool.tile([P, C], BF16, name=f"wv{i}") for i in range(ND)]
        for i in range(ND):
            dma(xq[i][:], xqT_d.ap()[i * P : (i + 1) * P, :])
            dma(xk[i][:], xkT_d.ap()[i * P : (i + 1) * P, :])
            dma(xv[i][:], xvT_d.ap()[i * P : (i + 1) * P, :])
            dma(wq[i][:], wqT_d.ap()[i * P : (i + 1) * P, :])
            dma(wk[i][:], wkT_d.ap()[i * P : (i + 1) * P, :])
            dma(wv[i][:], wvT_d.ap()[i * P : (i + 1) * P, :])
        prj = pctx.enter_context(tc.tile_pool(name="prjps", bufs=3, space="PSUM"))
        for cb in range(NCB):
            cslc = slice(cb * P, (cb + 1) * P)
            for sc in range(2):
                slc = slice(sc * 512, (sc + 1) * 512)
                ps_q = prj.tile([P, 512], FP32, tag="ps", name="ps_q")
                for db in range(ND):
                    nc.tensor.matmul(
                        ps_q[:],
                        lhsT=wq[db][:, cslc],
                        rhs=xq[db][:, slc],
                        start=(db == 0),
                        stop=False,
                    )
                nc.tensor.matmul(
                    ps_q[:],
                    lhsT=bqs[0:1, cslc],
                    rhs=ones[0:1, slc],
                    start=False,
                    stop=True,
                )
                nc.vector.tensor_copy(QT[cb][:, slc], ps_q[:])

                ps_k = prj.tile([P, 512], FP32, tag="ps", name="ps_k")
                for db in range(ND):
                    nc.tensor.matmul(
                        ps_k[:],
                        lhsT=wk[db][:, cslc],
                        rhs=xk[db][:, slc],
                        start=(db == 0),
                        stop=False,
                    )
                nc.tensor.matmul(
                    ps_k[:],
                    lhsT=bks[0:1, cslc],
                    rhs=ones[0:1, slc],
                    start=False,
                    stop=True,
                )
                nc.vector.tensor_copy(KT[cb][:, slc], ps_k[:])

        for sb in range(NS):
            ps_v = prj.tile([P, 512], FP32, tag="ps", name="ps_v")
            for db in range(ND):
                nc.tensor.matmul(
                    ps_v[:],
                    lhsT=xv[db][:, sb * P : (sb + 1) * P],
                    rhs=wv[db][:],
                    start=(db == 0),
                    stop=False,
                )
            nc.tensor.matmul(
                ps_v[:],
                lhsT=ones[0:1, sb * P : (sb + 1) * P],
                rhs=bvs[0:1, :],
                start=False,
                stop=True,
            )
            nc.vector.tensor_copy(V[sb][:], ps_v[:])

    # ---- attention, one head-pair (one QT/KT tile) at a time ----
    with ExitStack() as actx:
        psA = actx.enter_context(tc.tile_pool(name="psA", bufs=4, space="PSUM"))
        ppool = actx.enter_context(tc.tile_pool(name="pout", bufs=6))
        epool = actx.enter_context(tc.tile_pool(name="et", bufs=4))
        rpool = actx.enter_context(tc.tile_pool(name="rr", bufs=2))

        for hp in range(NCB):
            # phase A: S[q,k] = Q K^T; even/odd heads issued adjacently so
            # their K=64 matmuls pack onto disjoint PE row groups.
            for qb in range(NS):
                s_e = psA.tile([P, S], FP32, tag="s", name="s_e")
                s_o = psA.tile([P, S], FP32, tag="s", name="s_o")
                for jc in range(2):
                    slc = slice(jc * 512, (jc + 1) * 512)
                    for par, s_ps in ((0, s_e), (1, s_o)):
                        base = par * DH
                        nc.tensor.matmul(
                            s_ps[:, slc],
                            lhsT=QT[hp][base : base + DH, qb * P : (qb + 1) * P],
                            rhs=KT[hp][base : base + DH, slc],
                            start=True,
                            stop=True,
                        )
                for par, s_ps in ((0, s_e), (1, s_o)):
                    idx = (2 * hp + par) * NS + qb
                    p_t = ppool.tile([P, S], FP32, tag="p", name="p_t")
                    nc.scalar.activation(
                        p_t[:],
                        s_ps[:],
                        AF.Exp,
                        scale=SCALE,
                        accum_out=r_all[:, idx : idx + 1],
                    )
                    nc.vector.reciprocal(
                        rinv_all[:, idx : idx + 1], r_all[:, idx : idx + 1]
                    )
                    nc.vector.tensor_scalar_mul(
                        p_t[:], p_t[:], rinv_all[:, idx : idx + 1]
                    )
                    dma(
                        attn_d.ap()[2 * hp + par, qb * P : (qb + 1) * P, :],
                        p_t[:],
                    )

            # 1/rowsum rows -> broadcast across partitions via PE outer
            # product (even rows 0:64, odd rows 64:128 of one tile).
            rrows = []
            for par in range(2):
                h = 2 * hp + par
                rrow = rpool.tile([1, S], FP32, tag=f"rrow{par}", name=f"rrow{par}")
                for qb in range(NS):
                    idx = h * NS + qb
                    nc.gpsimd.dma_start(
                        rrow[0:1, qb * P : (qb + 1) * P],
                        rinv_all[:, idx : idx + 1],
                    )
                rrows.append(rrow)
            rbc_ps = psA.tile([P, S], FP32, tag="s", name="rbc_ps")
            for par in range(2):
                kwargs = {} if par == 0 else {"tile_position": (0, DH)}
                for jc in range(2):
                    slc = slice(jc * 512, (jc + 1) * 512)
                    nc.tensor.matmul(
                        rbc_ps[par * DH : (par + 1) * DH, slc],
                        lhsT=onesf[0:1, 0:DH],
                        rhs=rrows[par][0:1, slc],
                        start=True,
                        stop=True,
                        **kwargs,
                    )
            rbc = rpool.tile([P, S], FP32, tag="rbc", name="rbc")
            nc.vector.tensor_copy(rbc[:], rbc_ps[:])

            # phase B: ST[k,q] = K Q^T (packed pairs), exp -> bf16, then
            # OT = V^T @ exp(ST): even head accumulates into rows 0:64 of
            # otE, odd head into rows 64:128 of otO (col group 64) so the
            # evict multiply stays lane-aligned.
            otE = psA.tile([P, S], FP32, tag="s", name="otE")
            otO = psA.tile([P, S], FP32, tag="s", name="otO")
            for kb in range(NS):
                st_e = psA.tile([P, S], FP32, tag="s", name="st_e")
                st_o = psA.tile([P, S], FP32, tag="s", name="st_o")
                for jc in range(2):
                    slc = slice(jc * 512, (jc + 1) * 512)
                    for par, st in ((0, st_e), (1, st_o)):
                        base = par * DH
                        nc.tensor.matmul(
                            st[:, slc],
                            lhsT=KT[hp][base : base + DH, kb * P : (kb + 1) * P],
                            rhs=QT[hp][base : base + DH, slc],
                            start=True,
                            stop=True,
                        )
                et_e = epool.tile([P, S], BF16, tag="ete", name="et_e")
                et_o = epool.tile([P, S], BF16, tag="eto", name="et_o")
                nc.scalar.activation(et_e[:], st_e[:], AF.Exp, scale=SCALE)
                nc.scalar.activation(et_o[:], st_o[:], AF.Exp, scale=SCALE)
                for jc in range(2):
                    slc = slice(jc * 512, (jc + 1) * 512)
                    for par, et in ((0, et_e), (1, et_o)):
                        h = 2 * hp + par
                        if par == 0:
                            out_ap = otE[0:DH, slc]
                            kwargs = {}
                        else:
                            out_ap = otO[DH:P, slc]
                            kwargs = {"tile_position": (0, DH)}
                        nc.tensor.matmul(
                            out_ap,
                            lhsT=V[kb][:, h * DH : (h + 1) * DH],
                            rhs=et[:, slc],
                            start=(kb == 0),
                            stop=(kb == NS - 1),
                            **kwargs,
                        )
            nc.vector.tensor_mul(OT[hp][0:DH, :], otE[0:DH, :], rbc[0:DH, :])
            nc.vector.tensor_mul(OT[hp][DH:P, :], otO[DH:P, :], rbc[DH:P, :])

    # ---- output projection: partial[q, o] = OT^T @ woT (K=128 blocks) ----
    with ExitStack() as octx:
        opj = octx.enter_context(tc.tile_pool(name="opjps", bufs=2, space="PSUM"))
        osb = octx.enter_context(tc.tile_pool(name="osb", bufs=3))
        for qb in range(NS):
            for oc in range(2):
                slc = slice(oc * 512, (oc + 1) * 512)
                ps_o = opj.tile([P, 512], FP32, tag="po", name="ps_o")
                for cb in range(NCB):
                    nc.tensor.matmul(
                        ps_o[:],
                        lhsT=OT[cb][:, qb * P : (qb + 1) * P],
                        rhs=wo[cb][:, slc],
                        start=(cb == 0),
                        stop=(cb == NCB - 1),
                    )
                o_sb = osb.tile([P, 512], FP32, tag="o", name="o_sb")
                nc.vector.tensor_copy(o_sb[:], ps_o[:])
                dma(part_d.ap()[qb * P : (qb + 1) * P, slc], o_sb[:])


def _get_nc():
    if "nc" not in _CACHE:
        _CACHE["nc"] = _build_nc()
    return _CACHE["nc"]


def _prep_in_maps(q, k, v, Wq, bq, Wk, bk, Wv, bv, Wo):
    bf16 = ml_dtypes.bfloat16
    in_maps = []
    for c in range(8):
        b, g = divmod(c, 2)
        cs = slice(g * C, (g + 1) * C)
        in_maps.append(
            {
                "xqT": q[b].T.astype(bf16),
                "xkT": k[b].T.astype(bf16),
                "xvT": v[b].T.astype(bf16),
                "wqT": Wq[cs, :].T.astype(bf16),
                "wkT": Wk[cs, :].T.astype(bf16),
                "wvT": Wv[cs, :].T.astype(bf16),
                "bq": bq[cs].reshape(1, C).astype(bf16),
                "bk": bk[cs].reshape(1, C).astype(bf16),
                "bv": bv[cs].reshape(1, C).astype(bf16),
                "woT": Wo[:, cs].T.astype(bf16),
            }
        )
    return in_maps


def kernel(query, key_, value, Wq, bq, Wk, bk, Wv, bv, Wo, bo, _trace=False):
    from concourse.bass_utils import run_bass_kernel_spmd

    q = np.asarray(query, np.float32)
    k = np.asarray(key_, np.float32)
    v = np.asarray(value, np.float32)
    Wq = np.asarray(Wq, np.float32)
    bq = np.asarray(bq, np.float32)
    Wk = np.asarray(Wk, np.float32)
    bk = np.asarray(bk, np.float32)
    Wv = np.asarray(Wv, np.float32)
    bv = np.asarray(bv, np.float32)
    Wo = np.asarray(Wo, np.float32)
    bo = np.asarray(bo, np.float32)

    nc = _get_nc()
    in_maps = _prep_in_maps(q, k, v, Wq, bq, Wk, bk, Wv, bv, Wo)
    res = run_bass_kernel_spmd(nc, in_maps, list(range(8)), trace=_trace)
    _CACHE["last_result"] = res

    out = np.empty((B, S, D), np.float32)
    attention = np.empty((B, H, S, S), np.float32)
    for c in range(8):
        b, g = divmod(c, 2)
        attention[b, g * HPC : (g + 1) * HPC] = res.results[c]["attn"]
    for b in range(B):
        out[b] = (
            res.results[2 * b]["partial"]
            + res.results[2 * b + 1]["partial"]
            + bo[None, :]
        )
    return out, attention


# revision 33
# speedup vs baseline: 3.3687x; 1.0742x over previous
"""Trainium2 Bass kernel for 16-head MHA (B=4, S=1024, D=1024).

Returns (out, attention) matching the reference:
    Q/K/V = x @ W.T + b  (per-head split), scores = QK^T/sqrt(64),
    attention = softmax(scores), out = (attention @ V concat) @ Wo.T + bo.

Sharding: 8 cores = 4 batches x 2 head-groups (8 heads each).
Each core computes its batch's projections restricted to its 512
head-dims, full attention for its 8 heads, and a partial output
projection (contraction over its 512 head-dims). Host sums the two
partials per batch and adds bo.

Per-core layout (all matmul operands bf16, accumulation fp32):
  - inputs arrive transposed: xT[d, s] so projections need no on-chip
    transposes: QT/KT[c, s] (head-dim on partitions), V[s, c].
  - scores both ways: S[q, k] for the softmax/attention output (row
    softmax along free dim, exp row-sums via activation accum_out) and
    ST[k, q] for the P@V matmul (contraction needs k on partitions).
  - OT[c, q] = V^T @ exp(ST) accumulated over k blocks, normalized by
    broadcast 1/rowsum, then the output projection contracts c blocks.
"""

from contextlib import ExitStack

import ml_dtypes
import numpy as np

B = 4
S = 1024
D = 1024
H = 16
DH = 64
P = 128
C = 512  # head dims per core
HPC = 8  # heads per core
ND = D // P
NS = S // P
NCB = C // P
SCALE = 0.125  # 1 / sqrt(DH)

_CACHE = {}

# walrus in this environment rejects sequencer CTRL instructions (Drain/NoOp)
# carrying more than a couple of sem waits ("Too many sync wait commands").
# TileContext's tail drain waits on every active proc sem, so split it into a
# chain of drains with at most _MAX_CTRL_WAITS waits each.
_MAX_CTRL_WAITS = 1


def _patch_tail_drain():
    import concourse.mybir as mybir
    import concourse.tile as tile
    from concourse.vector_clock import ScopedClock

    if getattr(tile.TileContext._drain_and_barrier, "_split_waits", False):
        return

    def _drain_and_barrier(self, tick_clock, wait_clock):
        nc = self.nc
        drain_inst = nc.sync.drain()
        wait_clock.add_sem_waits(
            drain_inst.ins, ScopedClock({None: tick_clock.global_clock})
        )
        si = drain_inst.ins.sync_info
        waits = list(si.on_wait) if si is not None else []
        if len(waits) > _MAX_CTRL_WAITS:
            drain_inst.ins.sync_info = mybir.SyncInfo(
                on_wait=waits[:_MAX_CTRL_WAITS],
                on_update=list(si.on_update),
            )
            for i in range(_MAX_CTRL_WAITS, len(waits), _MAX_CTRL_WAITS):
                extra = nc.sync.drain()
                extra.ins.sync_info = mybir.SyncInfo(
                    on_wait=waits[i : i + _MAX_CTRL_WAITS], on_update=[]
                )
        nc.all_engine_barrier()
        assert self.sems is not None
        popped = nc._tile_sem_poison_stack.pop()
        assert popped is self._sem_poison
        nc.clear_and_free_semaphores(list(self.sems.allocated().values()))
        nc.all_engine_barrier()

    _drain_and_barrier._split_waits = True
    tile.TileContext._drain_and_barrier = _drain_and_barrier


def _split_sync_waits(nc, mybir, max_waits=1):
    """walrus here allows only one sync-wait command per instruction; hoist
    extra waits onto same-engine NoOps inserted just before the instruction
    (engine streams execute in block order, so this is equivalent)."""
    for fn in nc.m.functions:
        for blk in fn.blocks:
            out = []
            changed = False
            for inst in blk.instructions:
                si = inst.sync_info
                waits = list(si.on_wait) if si is not None else []
                if len(waits) > max_waits:
                    changed = True
                    for j, w in enumerate(waits[max_waits:]):
                        nop = mybir.InstNoOp(
                            name=f"{inst.name}-w{j}", ins=[], outs=[]
                        )
                        nop.engine = inst.engine
                        nop.sync_info = mybir.SyncInfo(on_wait=[w], on_update=[])
                        nc.register_instruction(nop, overwrite=True)
                        out.append(nop)
                    inst.sync_info = mybir.SyncInfo(
                        on_wait=waits[:max_waits], on_update=list(si.on_update)
                    )
                out.append(inst)
            if changed:
                blk.instructions = out


def _build_nc():
    import concourse.bass as bass
    import concourse.mybir as mybir
    import concourse.tile as tile

    _patch_tail_drain()

    FP32 = mybir.dt.float32
    BF16 = mybir.dt.bfloat16
    AF = mybir.ActivationFunctionType

    nc = bass.Bass("TRN2", target_bir_lowering=False, debug=False)

    xqT_d = nc.dram_tensor("xqT", [D, S], BF16, kind="ExternalInput")
    xkT_d = nc.dram_tensor("xkT", [D, S], BF16, kind="ExternalInput")
    xvT_d = nc.dram_tensor("xvT", [D, S], BF16, kind="ExternalInput")
    wqT_d = nc.dram_tensor("wqT", [D, C], BF16, kind="ExternalInput")
    wkT_d = nc.dram_tensor("wkT", [D, C], BF16, kind="ExternalInput")
    wvT_d = nc.dram_tensor("wvT", [D, C], BF16, kind="ExternalInput")
    bq_d = nc.dram_tensor("bq", [1, C], BF16, kind="ExternalInput")
    bk_d = nc.dram_tensor("bk", [1, C], BF16, kind="ExternalInput")
    bv_d = nc.dram_tensor("bv", [1, C], BF16, kind="ExternalInput")
    woT_d = nc.dram_tensor("woT", [C, D], BF16, kind="ExternalInput")
    attn_d = nc.dram_tensor("attn", [HPC, S, S], FP32, kind="ExternalOutput")
    part_d = nc.dram_tensor("partial", [S, D], FP32, kind="ExternalOutput")

    with tile.TileContext(nc) as tc:
        with ExitStack() as ctx:
            _body(
                ctx,
                tc,
                nc,
                FP32,
                BF16,
                AF,
                xqT_d,
                xkT_d,
                xvT_d,
                wqT_d,
                wkT_d,
                wvT_d,
                bq_d,
                bk_d,
                bv_d,
                woT_d,
                attn_d,
                part_d,
            )
    _split_sync_waits(nc, mybir)
    return nc


def _body(
    ctx,
    tc,
    nc,
    FP32,
    BF16,
    AF,
    xqT_d,
    xkT_d,
    xvT_d,
    wqT_d,
    wkT_d,
    wvT_d,
    bq_d,
    bk_d,
    bv_d,
    woT_d,
    attn_d,
    part_d,
):
    persist = ctx.enter_context(tc.tile_pool(name="persist", bufs=1))

    def ptile(name, shape, dt):
        return persist.tile(shape, dt, name=name)

    wo = [ptile(f"wo{i}", [P, D], BF16) for i in range(NCB)]
    bqs = ptile("bqs", [1, C], BF16)
    bks = ptile("bks", [1, C], BF16)
    bvs = ptile("bvs", [1, C], BF16)
    ones = ptile("ones", [1, S], BF16)
    onesf = ptile("onesf", [1, P], FP32)
    QT = [ptile(f"qt{i}", [P, S], BF16) for i in range(NCB)]
    KT = [ptile(f"kt{i}", [P, S], BF16) for i in range(NCB)]
    V = [ptile(f"v{i}", [P, C], BF16) for i in range(NS)]
    OT = [ptile(f"otc{i}", [P, S], BF16) for i in range(NCB)]
    r_all = ptile("r_all", [P, HPC * NS], FP32)
    rinv_all = ptile("rinv_all", [P, HPC * NS], FP32)

    dma = nc.sync.dma_start

    for i in range(NCB):
        dma(wo[i][:], woT_d.ap()[i * P : (i + 1) * P, :])
    dma(bqs[:], bq_d.ap()[:, :])
    dma(bks[:], bk_d.ap()[:, :])
    dma(bvs[:], bv_d.ap()[:, :])
    nc.vector.memset(ones[:], 1.0)
    nc.vector.memset(onesf[:], 1.0)

    # ---- projections: QT[c,s], KT[c,s] (transposed), V[s,c] (natural) ----
    with ExitStack() as pctx:
        xwpool = pctx.enter_context(tc.tile_pool(name="xw", bufs=1))
        xq = [xwpool.tile([P, S], BF16, name=f"xq{i}") for i in range(ND)]
        xk = [xwpool.tile([P, S], BF16, name=f"xk{i}") for i in range(ND)]
        xv = [xwpool.tile([P, S], BF16, name=f"xv{i}") for i in range(ND)]
        wq = [xwpool.tile([P, C], BF16, name=f"wq{i}") for i in range(ND)]
        wk = [xwpool.tile([P, C], BF16, name=f"wk{i}") for i in range(ND)]
        wv = [xwpool.tile([P, C], BF16, name=f"wv{i}") for i in range(ND)]
        for i in range(ND):
            dma(xq[i][:], xqT_d.ap()[i * P : (i + 1) * P, :])
            dma(xk[i][:], xkT_d.ap()[i * P : (i + 1) * P, :])
            dma(xv[i][:], xvT_d.ap()[i * P : (i + 1) * P, :])
            dma(wq[i][:], wqT_d.ap()[i * P : (i + 1) * P, :])
            dma(wk[i][:], wkT_d.ap()[i * P : (i + 1) * P, :])
            dma(wv[i][:], wvT_d.ap()[i * P : (i + 1) * P, :])
        prj = pctx.enter_context(tc.tile_pool(name="prjps", bufs=3, space="PSUM"))
        for cb in range(NCB):
            cslc = slice(cb * P, (cb + 1) * P)
            for sc in range(2):
                slc = slice(sc * 512, (sc + 1) * 512)
                ps_q = prj.tile([P, 512], FP32, tag="ps", name="ps_q")
                for db in range(ND):
                    nc.tensor.matmul(
                        ps_q[:],
                        lhsT=wq[db][:, cslc],
                        rhs=xq[db][:, slc],
                        start=(db == 0),
                        stop=False,
                    )
                nc.tensor.matmul(
                    ps_q[:],
                    lhsT=bqs[0:1, cslc],
                    rhs=ones[0:1, slc],
                    start=False,
                    stop=True,
                )
                nc.vector.tensor_copy(QT[cb][:, slc], ps_q[:])

                ps_k = prj.tile([P, 512], FP32, tag="ps", name="ps_k")
                for db in range(ND):
                    nc.tensor.matmul(
                        ps_k[:],
                        lhsT=wk[db][:, cslc],
                        rhs=xk[db][:, slc],
                        start=(db == 0),
                        stop=False,
                    )
                nc.tensor.matmul(
                    ps_k[:],
                    lhsT=bks[0:1, cslc],
                    rhs=ones[0:1, slc],
                    start=False,
                    stop=True,
                )
                nc.vector.tensor_copy(KT[cb][:, slc], ps_k[:])

        for sb in range(NS):
            ps_v = prj.tile([P, 512], FP32, tag="ps", name="ps_v")
            for db in range(ND):
                nc.tensor.matmul(
                    ps_v[:],
                    lhsT=xv[db][:, sb * P : (sb + 1) * P],
                    rhs=wv[db][:],
                    start=(db == 0),
                    stop=False,
                )
            nc.tensor.matmul(
                ps_v[:],
                lhsT=ones[0:1, sb * P : (sb + 1) * P],
                rhs=bvs[0:1, :],
                start=False,
                stop=True,
            )
            nc.vector.tensor_copy(V[sb][:], ps_v[:])

    # ---- attention, one head-pair (one QT/KT tile) at a time ----
    with ExitStack() as actx:
        psA = actx.enter_context(tc.tile_pool(name="psA", bufs=4, space="PSUM"))
        ppool = actx.enter_context(tc.tile_pool(name="pout", bufs=8))
        epool = actx.enter_context(tc.tile_pool(name="et", bufs=4))
        rpool = actx.enter_context(tc.tile_pool(name="rr", bufs=2))

        for hp in range(NCB):
            # phase A: S[q,k] = Q K^T; even/odd heads issued adjacently so
            # their K=64 matmuls pack onto disjoint PE row groups.
            for qb in range(NS):
                s_e = psA.tile([P, S], FP32, tag="s", name="s_e")
                s_o = psA.tile([P, S], FP32, tag="s", name="s_o")
                for jc in range(2):
                    slc = slice(jc * 512, (jc + 1) * 512)
                    for par, s_ps in ((0, s_e), (1, s_o)):
                        base = par * DH
                        nc.tensor.matmul(
                            s_ps[:, slc],
                            lhsT=QT[hp][base : base + DH, qb * P : (qb + 1) * P],
                            rhs=KT[hp][base : base + DH, slc],
                            start=True,
                            stop=True,
                        )
                for par, s_ps in ((0, s_e), (1, s_o)):
                    idx = (2 * hp + par) * NS + qb
                    p_t = ppool.tile([P, S], FP32, tag="p", name="p_t")
                    nc.scalar.activation(
                        p_t[:],
                        s_ps[:],
                        AF.Exp,
                        scale=SCALE,
                        accum_out=r_all[:, idx : idx + 1],
                    )
                    nc.vector.reciprocal(
                        rinv_all[:, idx : idx + 1], r_all[:, idx : idx + 1]
                    )
                    nc.vector.tensor_scalar_mul(
                        p_t[:], p_t[:], rinv_all[:, idx : idx + 1]
                    )
                    dma(
                        attn_d.ap()[2 * hp + par, qb * P : (qb + 1) * P, :],
                        p_t[:],
                    )

            # 1/rowsum rows -> broadcast across partitions via PE outer
            # product (even rows 0:64, odd rows 64:128 of one tile).
            rrows = []
            for par in range(2):
                h = 2 * hp + par
                rrow = rpool.tile([1, S], FP32, tag=f"rrow{par}", name=f"rrow{par}")
                for qb in range(NS):
                    idx = h * NS + qb
                    nc.gpsimd.dma_start(
                        rrow[0:1, qb * P : (qb + 1) * P],
                        rinv_all[:, idx : idx + 1],
                    )
                rrows.append(rrow)
            rbc_ps = psA.tile([P, S], FP32, tag="s", name="rbc_ps")
            for par in range(2):
                kwargs = {} if par == 0 else {"tile_position": (0, DH)}
                for jc in range(2):
                    slc = slice(jc * 512, (jc + 1) * 512)
                    nc.tensor.matmul(
                        rbc_ps[par * DH : (par + 1) * DH, slc],
                        lhsT=onesf[0:1, 0:DH],
                        rhs=rrows[par][0:1, slc],
                        start=True,
                        stop=True,
                        **kwargs,
                    )
            rbc = rpool.tile([P, S], FP32, tag="rbc", name="rbc")
            nc.vector.tensor_copy(rbc[:], rbc_ps[:])

            # phase B: ST[k,q] = K Q^T (packed pairs), exp -> bf16, then
            # OT = V^T @ exp(ST): even head accumulates into rows 0:64 of
            # otE, odd head into rows 64:128 of otO (col group 64) so the
            # evict multiply stays lane-aligned.
            oteo = psA.tile([P, S], FP32, tag="s", name="oteo")
            for kb in range(NS):
                st_e = psA.tile([P, S], FP32, tag="s", name="st_e")
                st_o = psA.tile([P, S], FP32, tag="s", name="st_o")
                for jc in range(2):
                    slc = slice(jc * 512, (jc + 1) * 512)
                    for par, st in ((0, st_e), (1, st_o)):
                        base = par * DH
                        nc.tensor.matmul(
                            st[:, slc],
                            lhsT=KT[hp][base : base + DH, kb * P : (kb + 1) * P],
                            rhs=QT[hp][base : base + DH, slc],
                            start=True,
                            stop=True,
                        )
                et_e = epool.tile([P, S], BF16, tag="ete", name="et_e")
                et_o = epool.tile([P, S], BF16, tag="eto", name="et_o")
                nc.scalar.activation(et_e[:], st_e[:], AF.Exp, scale=SCALE)
                nc.scalar.activation(et_o[:], st_o[:], AF.Exp, scale=SCALE)
                for jc in range(2):
                    slc = slice(jc * 512, (jc + 1) * 512)
                    for par, et in ((0, et_e), (1, et_o)):
                        h = 2 * hp + par
                        if par == 0:
                            out_ap = oteo[0:DH, slc]
                            kwargs = {}
                        else:
                            out_ap = oteo[DH:P, slc]
                            kwargs = {"tile_position": (0, DH)}
                        nc.tensor.matmul(
                            out_ap,
                            lhsT=V[kb][:, h * DH : (h + 1) * DH],
                            rhs=et[:, slc],
                            start=(kb == 0),
                            stop=(kb == NS - 1),
                            skip_group_check=True,
                            **kwargs,
                        )
            nc.vector.tensor_mul(OT[hp][0:DH, :], oteo[0:DH, :], rbc[0:DH, :])
            nc.vector.tensor_mul(OT[hp][DH:P, :], oteo[DH:P, :], rbc[DH:P, :])

    # ---- output projection: partial[q, o] = OT^T @ woT (K=128 blocks) ----
    with ExitStack() as octx:
        opj = octx.enter_context(tc.tile_pool(name="opjps", bufs=2, space="PSUM"))
        osb = octx.enter_context(tc.tile_pool(name="osb", bufs=3))
        for qb in range(NS):
            for oc in range(2):
                slc = slice(oc * 512, (oc + 1) * 512)
                ps_o = opj.tile([P, 512], FP32, tag="po", name="ps_o")
                for cb in range(NCB):
                    nc.tensor.matmul(
                        ps_o[:],
                        lhsT=OT[cb][:, qb * P : (qb + 1) * P],
                        rhs=wo[cb][:, slc],
                        start=(cb == 0),
                        stop=(cb == NCB - 1),
                    )
                o_sb = osb.tile([P, 512], FP32, tag="o", name="o_sb")
                nc.vector.tensor_copy(o_sb[:], ps_o[:])
                dma(part_d.ap()[qb * P : (qb + 1) * P, slc], o_sb[:])


def _get_nc():
    if "nc" not in _CACHE:
        _CACHE["nc"] = _build_nc()
    return _CACHE["nc"]


def _prep_in_maps(q, k, v, Wq, bq, Wk, bk, Wv, bv, Wo):
    bf16 = ml_dtypes.bfloat16
    in_maps = []
    for c in range(8):
        b, g = divmod(c, 2)
        cs = slice(g * C, (g + 1) * C)
        in_maps.append(
            {
                "xqT": q[b].T.astype(bf16),
                "xkT": k[b].T.astype(bf16),
                "xvT": v[b].T.astype(bf16),
                "wqT": Wq[cs, :].T.astype(bf16),
                "wkT": Wk[cs, :].T.astype(bf16),
                "wvT": Wv[cs, :].T.astype(bf16),
                "bq": bq[cs].reshape(1, C).astype(bf16),
                "bk": bk[cs].reshape(1, C).astype(bf16),
                "bv": bv[cs].reshape(1, C).astype(bf16),
                "woT": Wo[:, cs].T.astype(bf16),
            }
        )
    return in_maps


def kernel(query, key_, value, Wq, bq, Wk, bk, Wv, bv, Wo, bo, _trace=False):
    from concourse.bass_utils import run_bass_kernel_spmd

    q = np.asarray(query, np.float32)
    k = np.asarray(key_, np.float32)
    v = np.asarray(value, np.float32)
    Wq = np.asarray(Wq, np.float32)
    bq = np.asarray(bq, np.float32)
    Wk = np.asarray(Wk, np.float32)
    bk = np.asarray(bk, np.float32)
    Wv = np.asarray(Wv, np.float32)
    bv = np.asarray(bv, np.float32)
    Wo = np.asarray(Wo, np.float32)
    bo = np.asarray(bo, np.float32)

    nc = _get_nc()
    in_maps = _prep_in_maps(q, k, v, Wq, bq, Wk, bk, Wv, bv, Wo)
    res = run_bass_kernel_spmd(nc, in_maps, list(range(8)), trace=_trace)
    _CACHE["last_result"] = res

    out = np.empty((B, S, D), np.float32)
    attention = np.empty((B, H, S, S), np.float32)
    for c in range(8):
        b, g = divmod(c, 2)
        attention[b, g * HPC : (g + 1) * HPC] = res.results[c]["attn"]
    for b in range(B):
        out[b] = (
            res.results[2 * b]["partial"]
            + res.results[2 * b + 1]["partial"]
            + bo[None, :]
        )
    return out, attention
